# revision 1
# baseline (speedup 1.0000x reference)
"""Trainium2 Bass kernel for DGP-RFF + segment reduce (nn_DGP_RFF_36249523978481).

Data-parallel over 8 NeuronCores; per core two 31,250-point halves.
Host does index-only prep (argsort + greedy 128-seg-span tiling).
Device: dma_gather sorted x rows -> PE transpose -> RFF chain
(z0 fp32; z1/damp/mv bf16 matmuls; Sin/Exp/Ln on ACT, table-set batched)
-> per-tile one-hot matmul segment reduction -> SBUF accumulator
[128 values x 25128 segs] with register column offsets ->
ReduceScatter(add) over the 8 cores -> finalize -> host assembles.

Value layout (accumulator partition v): block b = v//16, r = v%16;
r<8: w-value 8b+r ; r>=8: inv-variance-value 8b+(r-8). ReduceScatter hands
core c partitions [16c,16c+16): rows 0:8 w / 8:16 inv for values 8c..8c+8.
"""
import sys
import numpy as np

sys.path.insert(0, "/opt/trn_rl_repo")

M = 500_000
NSEG = 25_000
NRFF = 256
NCORE = 8
MCORE = M // NCORE          # 62,500
MH = MCORE // 2             # 31,250 per half
NTH = 256                   # tiles per half (128 slots each)
NBH = NTH // 8              # gather batches per half (1024 idxs each) = 32
SBB = 4                     # gather batches per superbatch
NSB = NBH // SBB            # superbatches per half = 8
TCT = 512                   # compute-tile tokens
CPB = 1024 // TCT           # compute tiles per gather batch = 2
TRASH = NSEG
ACCF = NSEG + 128

_PROGRAM_CACHE = {}
DEBUG_SKIP_RS = False


def _val_perm():
    wcols = np.zeros(128, np.int64)
    is_w = np.zeros(128, np.bool_)
    for v in range(128):
        b, r = divmod(v, 16)
        wcols[v] = 8 * b + (r if r < 8 else r - 8)
        is_w[v] = r < 8
    return wcols, is_w


def _host_weights(Omega0, b0, Wm0, Wlv0, Omega1, b1, Wm1, Wlv1):
    c = np.sqrt(2.0 / NRFF)
    f64 = np.float64
    W01 = (c * (Wm0.astype(f64) @ Omega1.astype(f64))).astype(np.float32)
    V01 = (c * c * (np.exp(Wlv0.astype(f64)) @ (Omega1.astype(f64) ** 2))).astype(np.float32)
    Wm1p = (c * Wm1.astype(f64)).astype(np.float32)
    E1p = (c * c * np.exp(Wlv1.astype(f64))).astype(np.float32)
    wcols, is_w = _val_perm()
    MV = np.zeros((2 * NRFF, 128), np.float32)
    for v in range(128):
        k = wcols[v]
        if is_w[v]:
            MV[0:NRFF, v] = Wm1p[:, k]
        else:
            MV[NRFF:, v] = E1p[:, k]
    shift = np.pi / 2 + 64 * np.pi
    b0p = (np.asarray(b0, np.float64) + shift).astype(np.float32)[None, :]  # [1,256]
    b1p = (np.asarray(b1, np.float64) + shift).astype(np.float32)[None, :]
    return W01, V01, MV, b0p, b1p


def _host_tiles(idx_half):
    n = idx_half.shape[0]
    perm = np.argsort(idx_half, kind="stable")
    segs = idx_half[perm]
    gid = np.zeros(NTH * 128, np.int16)
    relv = np.full((NTH, 128), -1000.0, np.float32)
    offs = np.full(NTH, TRASH, np.int32)
    pos = 0
    t = 0
    while pos < n:
        if t >= NTH:
            raise RuntimeError("tile budget exceeded; increase NTH")
        base = segs[pos]
        end = int(min(np.searchsorted(segs, base + 128, side="left"),
                      pos + 128, n))
        cnt = end - pos
        gid[t * 128: t * 128 + cnt] = perm[pos:end].astype(np.int16)
        relv[t, :cnt] = (segs[pos:end] - base).astype(np.float32)
        offs[t] = base
        pos = end
        t += 1
    return gid, relv, offs


def _wrap_gidx(gid):
    out = np.zeros((NBH, 128, 64), np.int16)
    for b in range(NBH):
        blk = gid[b * 1024:(b + 1) * 1024].reshape(64, 16).T
        out[b] = np.tile(blk, (8, 1))
    return out


def _build_program():
    import concourse.bass as bass
    import concourse.bacc as bacc
    import concourse.mybir as mybir
    from concourse import tile
    from concourse.tile_rust import add_dep_helper

    f32 = mybir.dt.float32
    bf16 = mybir.dt.bfloat16
    i16 = mybir.dt.int16
    i32 = mybir.dt.int32
    AF = mybir.ActivationFunctionType
    ALU = mybir.AluOpType
    AX = mybir.AxisListType
    TWO_PI = float(2 * np.pi)
    INV2PI = float(1.0 / (2 * np.pi))
    MAGIC = 12582912.0

    # Restrict ACT table-set choice so ln/exp land in the combined
    # natural_log_exp set and sin in trig_and_small -> 2 loads per phase
    # switch instead of one per ln<->exp transition.
    import concourse.hw_specs as hw_specs
    _orig_gat = hw_specs.get_activation_tables
    AFt = mybir.ActivationFunctionType

    def _gat(arch):
        tabs = dict(_orig_gat(arch))
        keep = {"natural_log_exp_and_others", "trig_and_small"}
        out = {}
        for name, funcs in tabs.items():
            if name == "trig_and_small":
                out[name] = funcs
            elif name == "natural_log_exp_and_others":
                out[name] = funcs - {AFt.Sin}
            else:
                out[name] = funcs - {AFt.Sin, AFt.Exp, AFt.Ln, AFt.Square,
                                     AFt.Identity, AFt.Copy}
        return out

    bacc.get_activation_tables = _gat
    hw_specs.get_activation_tables = _gat

    nc = bacc.Bacc("TRN2", target_bir_lowering=False, debug=False,
                   num_devices=NCORE)

    _phase = {"acts": [], "anchor": None}

    def act(*args, **kw):
        inst = nc.scalar.activation(*args, **kw)
        if _phase["anchor"] is not None:
            add_dep_helper(inst.ins, _phase["anchor"], reason="act phase")
        _phase["acts"].append(inst.ins)
        return inst

    def act_phase():
        acts = _phase["acts"]
        if acts:
            anchor = acts[-1]
            for o in acts[:-1]:
                add_dep_helper(anchor, o, reason="act phase anchor")
            _phase["anchor"] = anchor
        _phase["acts"] = []

    xh = [nc.dram_tensor(f"xh{h}", [MH, 64], f32, kind="ExternalInput")
          for h in range(2)]
    gidx = nc.dram_tensor("gidx", [2, NBH, 128, 64], i16, kind="ExternalInput")
    relv = nc.dram_tensor("relv", [128, 2 * NTH], f32, kind="ExternalInput")
    offs = nc.dram_tensor("offs", [1, 2 * NTH], i32, kind="ExternalInput")
    om0 = nc.dram_tensor("om0", [64, NRFF], f32, kind="ExternalInput")
    w01 = nc.dram_tensor("w01", [NRFF, NRFF], bf16, kind="ExternalInput")
    v01 = nc.dram_tensor("v01", [NRFF, NRFF], bf16, kind="ExternalInput")
    wmv = nc.dram_tensor("wmv", [2 * NRFF, 128], bf16, kind="ExternalInput")
    b0p = nc.dram_tensor("b0p", [1, NRFF], f32, kind="ExternalInput")
    b1p = nc.dram_tensor("b1p", [1, NRFF], f32, kind="ExternalInput")
    iden = nc.dram_tensor("iden", [128, 128], f32, kind="ExternalInput")
    iotaf = nc.dram_tensor("iotaf", [128, 128], f32, kind="ExternalInput")

    accdump = (nc.dram_tensor("accdump", [128, NSEG], f32, kind="ExternalOutput")
               if DEBUG_SKIP_RS else None)
    ow = nc.dram_tensor("ow", [8, NSEG], f32, kind="ExternalOutput")
    ov = nc.dram_tensor("ov", [8, NSEG], f32, kind="ExternalOutput")

    CH = NSEG // 8  # 3125

    with tile.TileContext(nc, num_cores=NCORE) as tc:
        with (
            tc.tile_pool(name="const", bufs=1) as constp,
            tc.tile_pool(name="accp", bufs=1) as accp,
            tc.tile_pool(name="dram", bufs=1, space="DRAM") as dramp,
        ):
            om0_t = constp.tile([64, NRFF], f32, tag="om0")
            w01_t = constp.tile([128, 2, NRFF], bf16, tag="w01")
            v01_t = constp.tile([128, 2, NRFF], bf16, tag="v01")
            wmv_t = constp.tile([128, 4, 128], bf16, tag="wmv")
            b0_t = constp.tile([1, NRFF], f32, tag="b0")
            b1_t = constp.tile([1, NRFF], f32, tag="b1")
            id_t = constp.tile([128, 128], f32, tag="iden")
            io_t = constp.tile([128, 128], f32, tag="iota")
            rv_t = constp.tile([128, 2 * NTH], f32, tag="relv")
            of_t = constp.tile([1, 2 * NTH], i32, tag="offs")
            ones_t = constp.tile([1, TCT], f32, tag="ones")
            negpi_t = constp.tile([128, 1], f32, tag="negpi")
            magic_t = constp.tile([128, 1], f32, tag="magic")
            nc.sync.dma_start(om0_t[:], om0[:])
            nc.sync.dma_start(w01_t[:], w01[:].rearrange("(c p) n -> p c n", p=128))
            nc.sync.dma_start(v01_t[:], v01[:].rearrange("(c p) n -> p c n", p=128))
            nc.sync.dma_start(wmv_t[:], wmv[:].rearrange("(c p) n -> p c n", p=128))
            nc.sync.dma_start(b0_t[:], b0p[:])
            nc.sync.dma_start(b1_t[:], b1p[:])
            nc.sync.dma_start(id_t[:], iden[:])
            nc.sync.dma_start(io_t[:], iotaf[:])
            nc.sync.dma_start(rv_t[:], relv[:])
            nc.sync.dma_start(of_t[:], offs[:])
            nc.vector.memset(ones_t[:], 1.0)
            nc.vector.memset(negpi_t[:], -float(np.pi))
            nc.vector.memset(magic_t[:], MAGIC)

            acc_t = accp.tile([128, ACCF], f32, tag="acc")
            nc.vector.memset(acc_t[:], 0.0)

            regs = nc.alloc_registers("accoff",
                                      engines=[mybir.EngineType.DVE])
            off_rh = [r for r in regs][0]
            off_sv = nc.snap(off_rh, donate=True, min_val=0, max_val=ACCF - 128)

            with (
                tc.tile_pool(name="gath", bufs=3) as gathp,
                tc.tile_pool(name="stA", bufs=SBB * CPB + 1) as stA,
                tc.tile_pool(name="stS", bufs=CPB + 2) as stS,
                tc.tile_pool(name="work", bufs=2) as workp,
                tc.tile_pool(name="psum", bufs=2, space="PSUM") as psum,
            ):
                for h in range(2):
                    for sb in range(NSB):
                        xg_l, s1_l, sq0_l = [], [], []
                        for bi in range(SBB):
                            b = sb * SBB + bi
                            xg = gathp.tile([128, 8 * 64], f32, tag="xg")
                            gix = gathp.tile([128, 64], i16, tag="gix")
                            nc.sync.dma_start(gix[:], gidx[h, b, :, :])
                            nc.gpsimd.dma_gather(
                                xg[:].rearrange("p (c d) -> p c d", d=64),
                                xh[h][:], gix[:], 1024, 1024, 64)
                            xg_l.append(xg)
                        # ---- phase A (Sin) ----
                        for ct in range(SBB * CPB):
                            xg = xg_l[ct // CPB]
                            j0 = (ct % CPB) * 4
                            xTp = psum.tile([64, TCT], f32, tag="small")
                            for j in range(4):
                                nc.tensor.transpose(
                                    xTp[:, j * 128:(j + 1) * 128],
                                    xg[:, (j0 + j) * 64:(j0 + j + 1) * 64],
                                    id_t[:])
                            xT = workp.tile([64, TCT], f32, tag="xTs")
                            nc.vector.tensor_copy(xT[:], xTp[:])
                            z0 = psum.tile([128, 2, TCT], f32, tag="big")
                            for c in range(2):
                                nc.tensor.matmul(
                                    z0[:, c, :], om0_t[:, c * 128:(c + 1) * 128],
                                    xT[:], start=True, stop=False)
                                nc.tensor.matmul(
                                    z0[:, c, :], b0_t[:, c * 128:(c + 1) * 128],
                                    ones_t[:], start=False, stop=True)
                            rr0 = workp.tile([128, 2, TCT], f32, tag="rr0")
                            phi0 = workp.tile([128, 2, TCT], bf16, tag="phi0")
                            for c in range(2):
                                nc.vector.tensor_scalar(
                                    rr0[:, c, :], z0[:, c, :], INV2PI, MAGIC,
                                    ALU.mult, ALU.add)
                                nc.vector.tensor_scalar(
                                    rr0[:, c, :], rr0[:, c, :], MAGIC, -TWO_PI,
                                    ALU.subtract, ALU.mult)
                                nc.vector.tensor_add(
                                    rr0[:, c, :], rr0[:, c, :], z0[:, c, :])
                                act(phi0[:, c, :], rr0[:, c, :], AF.Sin)
                            sq0 = stA.tile([128, 2, TCT], bf16, tag="sq0")
                            for c in range(2):
                                nc.vector.tensor_mul(sq0[:, c, :], phi0[:, c, :], phi0[:, c, :])
                            z1 = psum.tile([128, 2, TCT], f32, tag="big")
                            for c in range(2):
                                for k in range(2):
                                    nc.tensor.matmul(
                                        z1[:, c, :],
                                        w01_t[:, k, c * 128:(c + 1) * 128],
                                        phi0[:, k, :], start=(k == 0),
                                        stop=False)
                                nc.tensor.matmul(
                                    z1[:, c, :], b1_t[:, c * 128:(c + 1) * 128],
                                    ones_t[:], start=False, stop=True)
                            rr1 = workp.tile([128, 2, TCT], f32, tag="rr1")
                            s1 = stA.tile([128, 2, TCT], bf16, tag="s1")
                            for c in range(2):
                                nc.vector.tensor_scalar(
                                    rr1[:, c, :], z1[:, c, :], INV2PI, MAGIC,
                                    ALU.mult, ALU.add)
                                nc.vector.tensor_scalar(
                                    rr1[:, c, :], rr1[:, c, :], MAGIC, -TWO_PI,
                                    ALU.subtract, ALU.mult)
                                nc.vector.tensor_add(
                                    rr1[:, c, :], rr1[:, c, :], z1[:, c, :])
                                act(s1[:, c, :], rr1[:, c, :], AF.Sin)
                            s1_l.append(s1)
                            sq0_l.append(sq0)
                        act_phase()
                        # ---- phase B (Ln/Exp) ----
                        for gb in range(SBB):
                            srcs = []
                            sscol = workp.tile([128, CPB * 4], f32, tag="ss")
                            for ci in range(CPB):
                                ct = gb * CPB + ci
                                s1 = s1_l[ct]
                                sq0 = sq0_l[ct]
                                damp = psum.tile([128, 2, TCT], f32, tag="big")
                                for c in range(2):
                                    for k in range(2):
                                        nc.tensor.matmul(
                                            damp[:, c, :],
                                            v01_t[:, k, c * 128:(c + 1) * 128],
                                            sq0[:, k, :], start=(k == 0),
                                            stop=(k == 1))
                                ed = workp.tile([128, 2, TCT], bf16, tag="ed")
                                for c in range(2):
                                    act(ed[:, c, :], damp[:, c, :],
                                        AF.Exp, scale=-0.5)
                                phi1 = workp.tile([128, 2, TCT], bf16, tag="phi1")
                                sq1 = workp.tile([128, 2, TCT], bf16, tag="sq1")
                                for c in range(2):
                                    nc.vector.tensor_mul(phi1[:, c, :], s1[:, c, :], ed[:, c, :])
                                    nc.vector.tensor_mul(sq1[:, c, :], phi1[:, c, :],
                                                         phi1[:, c, :])
                                mvp = psum.tile([128, TCT], f32, tag="mvp")
                                for j in range(TCT // 128):
                                    sl = slice(j * 128, (j + 1) * 128)
                                    nc.tensor.matmul(mvp[:, sl],
                                                     phi1[:, 0, sl], wmv_t[:, 0, :],
                                                     start=True, stop=False)
                                    nc.tensor.matmul(mvp[:, sl],
                                                     phi1[:, 1, sl], wmv_t[:, 1, :],
                                                     start=False, stop=False)
                                    nc.tensor.matmul(mvp[:, sl],
                                                     sq1[:, 0, sl], wmv_t[:, 2, :],
                                                     start=False, stop=False)
                                    nc.tensor.matmul(mvp[:, sl],
                                                     sq1[:, 1, sl], wmv_t[:, 3, :],
                                                     start=False, stop=True)
                                # views: [p, j, blk=8, r=16]
                                mv4 = mvp[:].rearrange(
                                    "p (j b r) -> p j b r", j=4, b=8)
                                lnv = workp.tile([128, 4, 8, 8], f32,
                                                 tag="lnv")
                                invv = workp.tile([128, 4, 8, 8], f32,
                                                  tag="invv")
                                act(lnv[:], mv4[:, :, :, 8:16], AF.Ln)
                                act(invv[:], lnv[:], AF.Exp, scale=-1.0)
                                src = stS.tile([128, 4, 128], f32, tag="src")
                                srcontainers = src[:].rearrange(
                                    "p j (b r) -> p j b r", b=8)
                                nc.vector.tensor_mul(
                                    srcontainers[:, :, :, 0:8],
                                    mv4[:, :, :, 0:8], invv[:])
                                nc.vector.tensor_copy(
                                    srcontainers[:, :, :, 8:16], invv[:])
                                m2sq = workp.tile([128, 4, 8, 8], f32,
                                                  tag="m2sq")
                                act(m2sq[:], mv4[:, :, :, 0:8], AF.Square)
                                nc.vector.tensor_reduce(
                                    sscol[:, ci * 4:(ci + 1) * 4], m2sq[:],
                                    AX.XY, ALU.add)
                                srcs.append(src)
                            lnss = workp.tile([128, CPB * 4], f32, tag="lnss")
                            rno = workp.tile([128, CPB * 4], f32, tag="rno")
                            act(lnss[:], sscol[:], AF.Ln)
                            act(rno[:], lnss[:], AF.Exp, scale=-0.5)
                            for ci in range(CPB):
                                ct = gb * CPB + ci
                                src = srcs[ci]
                                for j in range(4):
                                    t_idx = (h * NTH + sb * (SBB * 8)
                                             + ct * 4 + j)
                                    wview = src[:, j, :].rearrange(
                                        "p (b r) -> p b r", b=8)[:, :, 0:8]
                                    nc.vector.tensor_scalar_mul(
                                        wview, wview,
                                        rno[:, ci * 4 + j: ci * 4 + j + 1])
                                    oneh = workp.tile([128, 128], f32,
                                                      tag="oneh")
                                    nc.vector.tensor_scalar(
                                        oneh[:], io_t[:],
                                        rv_t[:, t_idx:t_idx + 1], None,
                                        ALU.is_equal)
                                    piece = psum.tile([128, 128], f32, tag="small")
                                    nc.tensor.matmul(piece[:], src[:, j, :],
                                                     oneh[:], start=True,
                                                     stop=True)
                                    nc.vector.reg_load(
                                        off_rh, of_t[:, t_idx:t_idx + 1])
                                    dst = acc_t[:, bass.ds(off_sv, 128)]
                                    nc.vector.tensor_add(dst, dst, piece[:])
                        act_phase()
                for h in range(2):
                    for sb in range(NSB):
                        xg_l, s1_l, sq0_l = [], [], []

            # ---- cross-core reduce + finalize ----
            accd = dramp.tile([128, NSEG], f32, tag="accd")
            rsod = dramp.tile([16, NSEG], f32, tag="rsod")
            nc.sync.dma_start(accd[:], acc_t[:, 0:NSEG])
            if DEBUG_SKIP_RS:
                nc.sync.dma_start(accdump[:], acc_t[:, 0:NSEG])
                nc.sync.dma_start(rsod[:], accd[0:16, :])
            else:
                nc.gpsimd.collective_compute(
                    "ReduceScatter", ALU.add,
                    replica_groups=[list(range(NCORE))],
                    ins=[accd.opt()], outs=[rsod.opt()])
            with tc.tile_pool(name="fin", bufs=1) as finp:
                rsw_t = finp.tile([64, CH], f32, tag="rsw")
                rsv_t = finp.tile([64, CH], f32, tag="rsv")
                nc.sync.dma_start(
                    rsw_t[:], rsod[0:8, :].rearrange("r (c s) -> (r c) s", s=CH))
                nc.sync.dma_start(
                    rsv_t[:], rsod[8:16, :].rearrange("r (c s) -> (r c) s", s=CH))
                lnv2 = finp.tile([64, CH], f32, tag="lnv2")
                ev_t = finp.tile([64, CH], f32, tag="ev")
                owt = finp.tile([64, CH], f32, tag="owt")
                act(lnv2[:], rsv_t[:], AF.Ln)
                act(ev_t[:], lnv2[:], AF.Exp, scale=-1.0)
                nc.vector.tensor_mul(owt[:], rsw_t[:], ev_t[:])
                nc.sync.dma_start(
                    ow[:].rearrange("r (c s) -> (r c) s", s=CH), owt[:])
                nc.sync.dma_start(
                    ov[:].rearrange("r (c s) -> (r c) s", s=CH), ev_t[:])
    nc.finalize()
    return nc


def _host_prep(x, idx, Omega0, b0, Wm0, Wlv0, Omega1, b1, Wm1, Wlv1):
    import ml_dtypes
    W01, V01, MV, b0p, b1p = _host_weights(Omega0, b0, Wm0, Wlv0,
                                           Omega1, b1, Wm1, Wlv1)
    w01b = W01.astype(ml_dtypes.bfloat16)
    v01b = V01.astype(ml_dtypes.bfloat16)
    mvb = MV.astype(ml_dtypes.bfloat16)
    iden = np.eye(128, dtype=np.float32)
    iota = np.tile(np.arange(128, dtype=np.float32)[None, :], (128, 1))
    om0f = np.ascontiguousarray(np.asarray(Omega0, np.float32))
    in_maps = []
    for c in range(NCORE):
        ic = idx[c * MCORE:(c + 1) * MCORE]
        xm = {"om0": om0f, "w01": w01b, "v01": v01b, "wmv": mvb,
              "b0p": b0p, "b1p": b1p, "iden": iden, "iotaf": iota}
        relv_all = np.zeros((128, 2 * NTH), np.float32)
        offs_all = np.zeros((1, 2 * NTH), np.int32)
        gidx_all = np.zeros((2, NBH, 128, 64), np.int16)
        for h in range(2):
            gid, relv, offs = _host_tiles(ic[h * MH:(h + 1) * MH])
            gidx_all[h] = _wrap_gidx(gid)
            relv_all[:, h * NTH:(h + 1) * NTH] = relv.T
            offs_all[0, h * NTH:(h + 1) * NTH] = offs
            xm[f"xh{h}"] = np.ascontiguousarray(
                x[c * MCORE + h * MH: c * MCORE + (h + 1) * MH])
        xm["relv"] = relv_all
        xm["offs"] = offs_all
        xm["gidx"] = gidx_all
        in_maps.append(xm)
    return in_maps


def kernel(x, Omega0, b0, Wm0, Wlv0, Omega1, b1, Wm1, Wlv1, x_idx):
    from concourse.bass_utils import run_bass_kernel_spmd

    x = np.ascontiguousarray(np.asarray(x, np.float32))
    idx = np.asarray(x_idx).astype(np.int64)
    in_maps = _host_prep(x, idx, np.asarray(Omega0), np.asarray(b0),
                         np.asarray(Wm0), np.asarray(Wlv0),
                         np.asarray(Omega1), np.asarray(b1),
                         np.asarray(Wm1), np.asarray(Wlv1))
    if "prog" not in _PROGRAM_CACHE:
        _PROGRAM_CACHE["prog"] = _build_program()
    nc = _PROGRAM_CACHE["prog"]
    res = run_bass_kernel_spmd(nc, in_maps, list(range(NCORE)))
    embed = np.zeros((NSEG, 64), np.float32)
    var = np.zeros((NSEG, 64), np.float32)
    for c in range(NCORE):
        embed[:, 8 * c: 8 * c + 8] = res.results[c]["ow"].T
        var[:, 8 * c: 8 * c + 8] = res.results[c]["ov"].T
    return embed, var



# revision 21
# speedup vs baseline: 1.1470x; 1.1470x over previous
"""Trainium2 Bass kernel for DGP-RFF + segment reduce (nn_DGP_RFF_36249523978481).

Data-parallel over 8 NeuronCores; per core two 31,250-point halves.
Host does index-only prep (argsort + greedy 128-seg-span tiling).
Device: dma_gather sorted x rows -> PE transpose -> RFF chain in u-space
(weights pre-scaled by 1/2pi on host; range reduction is a fused
(u+MAGIC)-MAGIC round + subtract on DVE, Sin runs with scale=2pi; z0 fp32,
z1/damp/mv bf16 matmuls; sq0/one-hot/normalize on the Pool engine)
-> per-tile one-hot matmul segment reduction -> SBUF accumulator
[128 values x 25128 segs] with register column offsets ->
ReduceScatter(add) over the 8 cores -> finalize -> host assembles.

Value layout (accumulator partition v): block b = v//16, r = v%16;
r<8: w-value 8b+r ; r>=8: inv-variance-value 8b+(r-8). ReduceScatter hands
core c partitions [16c,16c+16): rows 0:8 w / 8:16 inv for values 8c..8c+8.
"""
import sys
import numpy as np

sys.path.insert(0, "/opt/trn_rl_repo")

M = 500_000
NSEG = 25_000
NRFF = 256
NCORE = 8
MCORE = M // NCORE          # 62,500
MH = MCORE // 2             # 31,250 per half
NTH = 256                   # tiles per half (128 slots each)
NBH = NTH // 8              # gather batches per half (1024 idxs each) = 32
SBB = 4                     # gather batches per superbatch
NSB = NBH // SBB            # superbatches per half = 8
TCT = 512                   # compute-tile tokens
CPB = 1024 // TCT           # compute tiles per gather batch = 2
TRASH = NSEG
ACCF = NSEG + 128

_PROGRAM_CACHE = {}
DEBUG_SKIP_RS = False


def _val_perm():
    wcols = np.zeros(128, np.int64)
    is_w = np.zeros(128, np.bool_)
    for v in range(128):
        b, r = divmod(v, 16)
        wcols[v] = 8 * b + (r if r < 8 else r - 8)
        is_w[v] = r < 8
    return wcols, is_w


def _host_weights(Omega0, b0, Wm0, Wlv0, Omega1, b1, Wm1, Wlv1):
    # u-space: fold 1/(2*pi) into Omega0/W01/biases so the device range
    # reduction is f = u - round(u) and Sin runs with scale=2*pi.
    c = np.sqrt(2.0 / NRFF)
    f64 = np.float64
    INV2PI = 1.0 / (2 * np.pi)
    W01 = (c * INV2PI * (Wm0.astype(f64) @ Omega1.astype(f64))).astype(np.float32)
    V01 = (c * c * (np.exp(Wlv0.astype(f64)) @ (Omega1.astype(f64) ** 2))).astype(np.float32)
    Wm1p = (c * Wm1.astype(f64)).astype(np.float32)
    E1p = (c * c * np.exp(Wlv1.astype(f64))).astype(np.float32)
    wcols, is_w = _val_perm()
    MV = np.zeros((2 * NRFF, 128), np.float32)
    for v in range(128):
        k = wcols[v]
        if is_w[v]:
            MV[0:NRFF, v] = Wm1p[:, k]
        else:
            MV[NRFF:, v] = E1p[:, k]
    shift = np.pi / 2 + 64 * np.pi
    b0p = ((np.asarray(b0, np.float64) + shift) * INV2PI).astype(np.float32)[None, :]
    b1p = ((np.asarray(b1, np.float64) + shift) * INV2PI).astype(np.float32)[None, :]
    Om0u = (np.asarray(Omega0, f64) * INV2PI).astype(np.float32)
    return W01, V01, MV, b0p, b1p, Om0u


def _host_tiles(idx_half):
    n = idx_half.shape[0]
    perm = np.argsort(idx_half, kind="stable")
    segs = idx_half[perm]
    gid = np.zeros(NTH * 128, np.int16)
    relv = np.full((NTH, 128), -1000.0, np.float32)
    offs = np.full(NTH, TRASH, np.int32)
    pos = 0
    t = 0
    while pos < n:
        if t >= NTH:
            raise RuntimeError("tile budget exceeded; increase NTH")
        base = segs[pos]
        end = int(min(np.searchsorted(segs, base + 128, side="left"),
                      pos + 128, n))
        cnt = end - pos
        gid[t * 128: t * 128 + cnt] = perm[pos:end].astype(np.int16)
        relv[t, :cnt] = (segs[pos:end] - base).astype(np.float32)
        offs[t] = base
        pos = end
        t += 1
    return gid, relv, offs


def _wrap_gidx(gid):
    out = np.zeros((NBH, 128, 64), np.int16)
    for b in range(NBH):
        blk = gid[b * 1024:(b + 1) * 1024].reshape(64, 16).T
        out[b] = np.tile(blk, (8, 1))
    return out


def _build_program():
    import concourse.bass as bass
    import concourse.bacc as bacc
    import concourse.mybir as mybir
    from concourse import tile
    from concourse.tile_rust import add_dep_helper

    f32 = mybir.dt.float32
    f32r = mybir.dt.float32r
    bf16 = mybir.dt.bfloat16
    i16 = mybir.dt.int16
    i32 = mybir.dt.int32
    AF = mybir.ActivationFunctionType
    ALU = mybir.AluOpType
    AX = mybir.AxisListType
    TWO_PI = float(2 * np.pi)
    INV2PI = float(1.0 / (2 * np.pi))
    MAGIC = 12582912.0

    # Restrict ACT table-set choice so ln/exp land in the combined
    # natural_log_exp set and sin in trig_and_small -> 2 loads per phase
    # switch instead of one per ln<->exp transition.
    import concourse.hw_specs as hw_specs
    _orig_gat = hw_specs.get_activation_tables
    AFt = mybir.ActivationFunctionType

    def _gat(arch):
        tabs = dict(_orig_gat(arch))
        keep = {"natural_log_exp_and_others", "trig_and_small"}
        out = {}
        for name, funcs in tabs.items():
            if name == "trig_and_small":
                out[name] = funcs
            elif name == "natural_log_exp_and_others":
                out[name] = funcs - {AFt.Sin}
            else:
                out[name] = funcs - {AFt.Sin, AFt.Exp, AFt.Ln, AFt.Square,
                                     AFt.Identity, AFt.Copy}
        return out

    bacc.get_activation_tables = _gat
    hw_specs.get_activation_tables = _gat

    nc = bacc.Bacc("TRN2", target_bir_lowering=False, debug=False,
                   num_devices=NCORE)

    _phase = {"acts": [], "anchor": None}

    def act(*args, **kw):
        inst = nc.scalar.activation(*args, **kw)
        if _phase["anchor"] is not None:
            add_dep_helper(inst.ins, _phase["anchor"], reason="act phase")
        _phase["acts"].append(inst.ins)
        return inst

    def act_phase():
        acts = _phase["acts"]
        if acts:
            anchor = acts[-1]
            for o in acts[:-1]:
                add_dep_helper(anchor, o, reason="act phase anchor")
            _phase["anchor"] = anchor
        _phase["acts"] = []

    xh = [nc.dram_tensor(f"xh{h}", [MH, 64], f32, kind="ExternalInput")
          for h in range(2)]
    gidx = nc.dram_tensor("gidx", [2, NBH, 128, 64], i16, kind="ExternalInput")
    relv = nc.dram_tensor("relv", [128, 2 * NTH], f32, kind="ExternalInput")
    offs = nc.dram_tensor("offs", [1, 2 * NTH], i32, kind="ExternalInput")
    om0 = nc.dram_tensor("om0", [64, NRFF], f32, kind="ExternalInput")
    w01 = nc.dram_tensor("w01", [NRFF, NRFF], bf16, kind="ExternalInput")
    v01 = nc.dram_tensor("v01", [NRFF, NRFF], bf16, kind="ExternalInput")
    wmv = nc.dram_tensor("wmv", [2 * NRFF, 128], bf16, kind="ExternalInput")
    b0p = nc.dram_tensor("b0p", [1, NRFF], f32, kind="ExternalInput")
    b1p = nc.dram_tensor("b1p", [1, NRFF], f32, kind="ExternalInput")
    iden = nc.dram_tensor("iden", [128, 128], f32, kind="ExternalInput")
    iotaf = nc.dram_tensor("iotaf", [128, 128], f32, kind="ExternalInput")
    onesd = nc.dram_tensor("onesd", [1, TCT], f32, kind="ExternalInput")

    accdump = (nc.dram_tensor("accdump", [128, NSEG], f32, kind="ExternalOutput")
               if DEBUG_SKIP_RS else None)
    ow = nc.dram_tensor("ow", [8, NSEG], f32, kind="ExternalOutput")
    ov = nc.dram_tensor("ov", [8, NSEG], f32, kind="ExternalOutput")

    CH = NSEG // 8  # 3125

    with tile.TileContext(nc, num_cores=NCORE) as tc:
        with (
            tc.tile_pool(name="const", bufs=1) as constp,
            tc.tile_pool(name="accp", bufs=1) as accp,
            tc.tile_pool(name="dram", bufs=1, space="DRAM") as dramp,
        ):
            om0_t = constp.tile([64, NRFF], f32, tag="om0")
            w01_t = constp.tile([128, 2, NRFF], bf16, tag="w01")
            v01_t = constp.tile([128, 2, NRFF], bf16, tag="v01")
            wmv_t = constp.tile([128, 4, 128], bf16, tag="wmv")
            b0_t = constp.tile([1, NRFF], f32, tag="b0")
            b1_t = constp.tile([1, NRFF], f32, tag="b1")
            id_t = constp.tile([128, 128], f32, tag="iden")
            io_t = constp.tile([128, 128], f32, tag="iota")
            rv_t = constp.tile([128, 2 * NTH], f32, tag="relv")
            of_t = constp.tile([1, 2 * NTH], i32, tag="offs")
            ones_t = constp.tile([1, TCT], f32, tag="ones")
            negpi_t = constp.tile([128, 1], f32, tag="negpi")
            magic_t = constp.tile([128, 1], f32, tag="magic")
            nc.sync.dma_start(om0_t[:], om0[:])
            nc.sync.dma_start(w01_t[:], w01[:].rearrange("(c p) n -> p c n", p=128))
            nc.sync.dma_start(v01_t[:], v01[:].rearrange("(c p) n -> p c n", p=128))
            nc.sync.dma_start(wmv_t[:], wmv[:].rearrange("(c p) n -> p c n", p=128))
            nc.sync.dma_start(b0_t[:], b0p[:])
            nc.sync.dma_start(b1_t[:], b1p[:])
            nc.sync.dma_start(id_t[:], iden[:])
            nc.sync.dma_start(io_t[:], iotaf[:])
            nc.sync.dma_start(rv_t[:], relv[:])
            nc.sync.dma_start(of_t[:], offs[:])
            nc.sync.dma_start(ones_t[:], onesd[:])
            nc.vector.memset(negpi_t[:], -float(np.pi))
            nc.vector.memset(magic_t[:], MAGIC)

            acc_t = accp.tile([128, ACCF], f32, tag="acc")
            nc.vector.memset(acc_t[:], 0.0)

            regs = nc.alloc_registers("accoff",
                                      engines=[mybir.EngineType.DVE])
            off_rh = [r for r in regs][0]
            off_sv = nc.snap(off_rh, donate=True, min_val=0, max_val=ACCF - 128)

            with (
                tc.tile_pool(name="gath", bufs=3) as gathp,
                tc.tile_pool(name="stA", bufs=SBB * CPB + 1) as stA,
                tc.tile_pool(name="stS", bufs=CPB + 2) as stS,
                tc.tile_pool(name="work", bufs=2) as workp,
                tc.tile_pool(name="psum", bufs=2, space="PSUM") as psum,
            ):
                for h in range(2):
                    for sb in range(NSB):
                        xg_l, s1_l, sq0_l = [], [], []
                        for bi in range(SBB):
                            b = sb * SBB + bi
                            xg = gathp.tile([128, 8 * 64], f32, tag="xg")
                            gix = gathp.tile([128, 64], i16, tag="gix")
                            nc.sync.dma_start(gix[:], gidx[h, b, :, :])
                            nc.gpsimd.dma_gather(
                                xg[:].rearrange("p (c d) -> p c d", d=64),
                                xh[h][:], gix[:], 1024, 1024, 64)
                            xg_l.append(xg)
                        # ---- phase A (Sin) ----
                        for ct in range(SBB * CPB):
                            xg = xg_l[ct // CPB]
                            j0 = (ct % CPB) * 4
                            xTp = psum.tile([64, TCT], f32, tag="small")
                            for j in range(4):
                                nc.tensor.transpose(
                                    xTp[:, j * 128:(j + 1) * 128],
                                    xg[:, (j0 + j) * 64:(j0 + j + 1) * 64],
                                    id_t[:])
                            xT = workp.tile([64, TCT], f32, tag="xTs")
                            nc.scalar.copy(xT[:], xTp[:])
                            z0 = psum.tile([128, 2, TCT], f32, tag="big")
                            for c in range(2):
                                nc.tensor.matmul(
                                    z0[:, c, :], om0_t[:, c * 128:(c + 1) * 128],
                                    xT[:], start=True, stop=False)
                                nc.tensor.matmul(
                                    z0[:, c, :], b0_t[:, c * 128:(c + 1) * 128],
                                    ones_t[:], start=False, stop=True)
                            rr0 = workp.tile([128, 2, TCT], f32, tag="rr0")
                            phi0 = workp.tile([128, 2, TCT], bf16, tag="phi0")
                            # k = round(u) via magic add/sub (fused), f = u - k
                            for c in range(2):
                                nc.vector.tensor_scalar(
                                    rr0[:, c, :], z0[:, c, :], MAGIC, MAGIC,
                                    ALU.add, ALU.subtract)
                                nc.vector.tensor_sub(
                                    rr0[:, c, :], z0[:, c, :], rr0[:, c, :])
                                act(phi0[:, c, :], rr0[:, c, :], AF.Sin,
                                    scale=TWO_PI)
                            sq0 = stA.tile([128, 2, TCT], bf16, tag="sq0")
                            for c in range(2):
                                nc.gpsimd.tensor_mul(sq0[:, c, :], phi0[:, c, :], phi0[:, c, :])
                            z1 = psum.tile([128, 2, TCT], f32, tag="big")
                            for c in range(2):
                                for k in range(2):
                                    nc.tensor.matmul(
                                        z1[:, c, :],
                                        w01_t[:, k, c * 128:(c + 1) * 128],
                                        phi0[:, k, :], start=(k == 0),
                                        stop=False)
                                nc.tensor.matmul(
                                    z1[:, c, :], b1_t[:, c * 128:(c + 1) * 128],
                                    ones_t[:], start=False, stop=True)
                            rr1 = workp.tile([128, 2, TCT], f32, tag="rr1")
                            s1 = stA.tile([128, 2, TCT], bf16, tag="s1")
                            for c in range(2):
                                nc.vector.tensor_scalar(
                                    rr1[:, c, :], z1[:, c, :], MAGIC, MAGIC,
                                    ALU.add, ALU.subtract)
                                nc.vector.tensor_sub(
                                    rr1[:, c, :], z1[:, c, :], rr1[:, c, :])
                                act(s1[:, c, :], rr1[:, c, :], AF.Sin,
                                    scale=TWO_PI)
                            s1_l.append(s1)
                            sq0_l.append(sq0)
                        act_phase()
                        # ---- phase B (Ln/Exp) ----
                        for gb in range(SBB):
                            srcs = []
                            sscol = workp.tile([128, CPB * 4], f32, tag="ss")
                            for ci in range(CPB):
                                ct = gb * CPB + ci
                                s1 = s1_l[ct]
                                sq0 = sq0_l[ct]
                                damp = psum.tile([128, 2, TCT], f32, tag="big")
                                for c in range(2):
                                    for k in range(2):
                                        nc.tensor.matmul(
                                            damp[:, c, :],
                                            v01_t[:, k, c * 128:(c + 1) * 128],
                                            sq0[:, k, :], start=(k == 0),
                                            stop=(k == 1))
                                ed = workp.tile([128, 2, TCT], bf16, tag="ed")
                                act(ed[:], damp[:], AF.Exp, scale=-0.5)
                                phi1 = workp.tile([128, 2, TCT], bf16, tag="phi1")
                                sq1 = workp.tile([128, 2, TCT], bf16, tag="sq1")
                                nc.vector.tensor_mul(phi1[:], s1[:], ed[:])
                                nc.vector.tensor_mul(sq1[:], phi1[:], phi1[:])
                                mvp = psum.tile([128, TCT], f32, tag="mvp")
                                for j in range(TCT // 128):
                                    sl = slice(j * 128, (j + 1) * 128)
                                    nc.tensor.matmul(mvp[:, sl],
                                                     phi1[:, 0, sl], wmv_t[:, 0, :],
                                                     start=True, stop=False)
                                    nc.tensor.matmul(mvp[:, sl],
                                                     phi1[:, 1, sl], wmv_t[:, 1, :],
                                                     start=False, stop=False)
                                    nc.tensor.matmul(mvp[:, sl],
                                                     sq1[:, 0, sl], wmv_t[:, 2, :],
                                                     start=False, stop=False)
                                    nc.tensor.matmul(mvp[:, sl],
                                                     sq1[:, 1, sl], wmv_t[:, 3, :],
                                                     start=False, stop=True)
                                # views: [p, j, blk=8, r=16]
                                mv4 = mvp[:].rearrange(
                                    "p (j b r) -> p j b r", j=4, b=8)
                                lnv = workp.tile([128, 4, 8, 8], f32,
                                                 tag="lnv")
                                invv = workp.tile([128, 4, 8, 8], f32,
                                                  tag="invv")
                                act(lnv[:], mv4[:, :, :, 8:16], AF.Ln)
                                act(invv[:], lnv[:], AF.Exp, scale=-1.0)
                                src = stS.tile([128, 4, 128], f32, tag="src")
                                srcontainers = src[:].rearrange(
                                    "p j (b r) -> p j b r", b=8)
                                nc.vector.tensor_mul(
                                    srcontainers[:, :, :, 0:8],
                                    mv4[:, :, :, 0:8], invv[:])
                                nc.vector.tensor_copy(
                                    srcontainers[:, :, :, 8:16], invv[:])
                                m2sq = workp.tile([128, 4, 8, 8], f32,
                                                  tag="m2sq")
                                act(m2sq[:], mv4[:, :, :, 0:8], AF.Square)
                                nc.vector.tensor_reduce(
                                    sscol[:, ci * 4:(ci + 1) * 4], m2sq[:],
                                    AX.XY, ALU.add)
                                srcs.append(src)
                            lnss = workp.tile([128, CPB * 4], f32, tag="lnss")
                            rno = workp.tile([128, CPB * 4], f32, tag="rno")
                            act(lnss[:], sscol[:], AF.Ln)
                            act(rno[:], lnss[:], AF.Exp, scale=-0.5)
                            for ci in range(CPB):
                                ct = gb * CPB + ci
                                src = srcs[ci]
                                for j in range(4):
                                    t_idx = (h * NTH + sb * (SBB * 8)
                                             + ct * 4 + j)
                                    wview = src[:, j, :].rearrange(
                                        "p (b r) -> p b r", b=8)[:, :, 0:8]
                                    nc.gpsimd.tensor_scalar_mul(
                                        wview, wview,
                                        rno[:, ci * 4 + j: ci * 4 + j + 1])
                                    oneh = workp.tile([128, 128], f32,
                                                      tag="oneh")
                                    nc.gpsimd.tensor_scalar(
                                        oneh[:], io_t[:],
                                        rv_t[:, t_idx:t_idx + 1], None,
                                        ALU.is_equal)
                                    piece = psum.tile([128, 128], f32, tag="small")
                                    nc.tensor.matmul(piece[:], src[:, j, :],
                                                     oneh[:], start=True,
                                                     stop=True)
                                    nc.vector.reg_load(
                                        off_rh, of_t[:, t_idx:t_idx + 1])
                                    dst = acc_t[:, bass.ds(off_sv, 128)]
                                    nc.vector.tensor_add(dst, dst, piece[:])
                        act_phase()
                for h in range(2):
                    for sb in range(NSB):
                        xg_l, s1_l, sq0_l = [], [], []

            # ---- cross-core reduce + finalize ----
            accd = dramp.tile([128, NSEG], f32, tag="accd")
            rsod = dramp.tile([16, NSEG], f32, tag="rsod")
            nc.sync.dma_start(accd[:], acc_t[:, 0:NSEG])
            if DEBUG_SKIP_RS:
                nc.sync.dma_start(accdump[:], acc_t[:, 0:NSEG])
                nc.sync.dma_start(rsod[:], accd[0:16, :])
            else:
                nc.gpsimd.collective_compute(
                    "ReduceScatter", ALU.add,
                    replica_groups=[list(range(NCORE))],
                    ins=[accd.opt()], outs=[rsod.opt()])
            with tc.tile_pool(name="fin", bufs=1) as finp:
                rsw_t = finp.tile([64, CH], f32, tag="rsw")
                rsv_t = finp.tile([64, CH], f32, tag="rsv")
                nc.sync.dma_start(
                    rsw_t[:], rsod[0:8, :].rearrange("r (c s) -> (r c) s", s=CH))
                nc.sync.dma_start(
                    rsv_t[:], rsod[8:16, :].rearrange("r (c s) -> (r c) s", s=CH))
                lnv2 = finp.tile([64, CH], f32, tag="lnv2")
                ev_t = finp.tile([64, CH], f32, tag="ev")
                owt = finp.tile([64, CH], f32, tag="owt")
                act(lnv2[:], rsv_t[:], AF.Ln)
                act(ev_t[:], lnv2[:], AF.Exp, scale=-1.0)
                nc.vector.tensor_mul(owt[:], rsw_t[:], ev_t[:])
                nc.sync.dma_start(
                    ow[:].rearrange("r (c s) -> (r c) s", s=CH), owt[:])
                nc.sync.dma_start(
                    ov[:].rearrange("r (c s) -> (r c) s", s=CH), ev_t[:])
    nc.finalize()
    return nc


def _host_prep(x, idx, Omega0, b0, Wm0, Wlv0, Omega1, b1, Wm1, Wlv1):
    import ml_dtypes
    W01, V01, MV, b0p, b1p, Om0u = _host_weights(Omega0, b0, Wm0, Wlv0,
                                                 Omega1, b1, Wm1, Wlv1)
    w01b = W01.astype(ml_dtypes.bfloat16)
    v01b = V01.astype(ml_dtypes.bfloat16)
    mvb = MV.astype(ml_dtypes.bfloat16)
    iden = np.eye(128, dtype=np.float32)
    iota = np.tile(np.arange(128, dtype=np.float32)[None, :], (128, 1))
    om0f = np.ascontiguousarray(Om0u)
    in_maps = []
    for c in range(NCORE):
        ic = idx[c * MCORE:(c + 1) * MCORE]
        xm = {"om0": om0f, "w01": w01b, "v01": v01b, "wmv": mvb,
              "b0p": b0p, "b1p": b1p, "iden": iden, "iotaf": iota,
              "onesd": np.ones((1, TCT), np.float32)}
        relv_all = np.zeros((128, 2 * NTH), np.float32)
        offs_all = np.zeros((1, 2 * NTH), np.int32)
        gidx_all = np.zeros((2, NBH, 128, 64), np.int16)
        for h in range(2):
            gid, relv, offs = _host_tiles(ic[h * MH:(h + 1) * MH])
            gidx_all[h] = _wrap_gidx(gid)
            relv_all[:, h * NTH:(h + 1) * NTH] = relv.T
            offs_all[0, h * NTH:(h + 1) * NTH] = offs
            xm[f"xh{h}"] = np.ascontiguousarray(
                x[c * MCORE + h * MH: c * MCORE + (h + 1) * MH])
        xm["relv"] = relv_all
        xm["offs"] = offs_all
        xm["gidx"] = gidx_all
        in_maps.append(xm)
    return in_maps


def kernel(x, Omega0, b0, Wm0, Wlv0, Omega1, b1, Wm1, Wlv1, x_idx):
    from concourse.bass_utils import run_bass_kernel_spmd

    x = np.ascontiguousarray(np.asarray(x, np.float32))
    idx = np.asarray(x_idx).astype(np.int64)
    in_maps = _host_prep(x, idx, np.asarray(Omega0), np.asarray(b0),
                         np.asarray(Wm0), np.asarray(Wlv0),
                         np.asarray(Omega1), np.asarray(b1),
                         np.asarray(Wm1), np.asarray(Wlv1))
    if "prog" not in _PROGRAM_CACHE:
        _PROGRAM_CACHE["prog"] = _build_program()
    nc = _PROGRAM_CACHE["prog"]
    res = run_bass_kernel_spmd(nc, in_maps, list(range(NCORE)))
    embed = np.zeros((NSEG, 64), np.float32)
    var = np.zeros((NSEG, 64), np.float32)
    for c in range(NCORE):
        embed[:, 8 * c: 8 * c + 8] = res.results[c]["ow"].T
        var[:, 8 * c: 8 * c + 8] = res.results[c]["ov"].T
    return embed, var

